# revision 55
# baseline (speedup 1.0000x reference)
"""GQA attention block (16 query heads / 4 KV groups, head_dim 128) on 8 TRN2 NeuronCores.

Sharding: data-parallel over batch (b=2) x tensor-parallel over the 4 KV groups.
Core c handles batch c//4, KV group c%4 (4 query heads). Each core computes its
group's Q/K/V projections, causal softmax attention, and a partial out-projection
(row-shard of Wo); the host sums the 4 partials per batch and adds the bias.

All matmuls run in bf16 (fp32 PSUM accumulation). Host pre-transposes x to x^T
(and packs Wk/Wv partition-major) so every matmul operand is already in the
[K, M]/[K, N] layout the PE wants; the only on-chip transposes are the per-block
128x128 context-tile transposes ahead of the out-projection. Softmax runs without
the running-max (score scale is bounded by the input distribution); the denominator
comes from a ones-column appended to V.

Schedule highlights:
- Startup DMAs rotate wk/wq(h0,h1)/x chunks in exact consumption order so
  K, Q(h0), Q(h1) stream behind the DMA from ~3us; dummy identity matmuls
  warm the PE p-state during the initial DMA latency so real matmuls run at
  full clock (the cost model halves PE speed until ~3us of sustained use).
- Scores for the two heads of a GQA head-pair go into one 2-bank PSUM tile
  [128, 2, 512] and share a single Exp activation per k-tile: 80 exps total
  instead of 160 (the Act engine paced the attention endgame before).
- Projection and out-projection groups share one 2-buf PSUM tag, and the
  out-projections of block N are woven into block N+1's attention emission:
  ring order matches temporal order while the scheduler's ready-heap pops
  them exactly in the Act-paced stalls. The last block's out-projections are
  emitted per sub-tile right behind the softmax chains, with a low-latency
  DVE-copy + per-chunk-DMA path on the final row tile to shorten the drain.
"""

import math
import os

import ml_dtypes
import numpy as np

B = 2
T = 2048
D_IN = 2048
N_KV = 4          # KV groups (one per core within a batch)
GH = 4            # query heads per KV group
HD = 128          # head dim
GD = GH * HD      # 512: per-group q/ctx width
TT = T // 128     # 16 row tiles
CC = D_IN // 128  # 16 contraction chunks
NQ = T // 512     # 4 query chunks of 512
SCALE = 1.0 / math.sqrt(HD)

CFG_PTP = int(os.environ.get("K_PTP", "22"))
CFG_WARM = int(os.environ.get("K_WARM", "14"))

_COMPILED = None


def _build():
    import concourse.bacc as bacc
    import concourse.tile as tile
    from concourse import mybir
    from concourse.masks import make_identity

    bf16 = mybir.dt.bfloat16
    f32 = mybir.dt.float32

    nc = bacc.Bacc("TRN2", target_bir_lowering=False, debug=False)

    # xT: x^T per batch; wk/wv packed partition-major on host: [128, c*HD]
    xT_d = nc.dram_tensor("xT", [D_IN, T], bf16, kind="ExternalInput")
    wq_d = nc.dram_tensor("wq", [D_IN, GD], bf16, kind="ExternalInput")
    wk_d = nc.dram_tensor("wk", [128, CC * HD], bf16, kind="ExternalInput")
    wv_d = nc.dram_tensor("wv", [128, CC * HD], bf16, kind="ExternalInput")
    wo_d = nc.dram_tensor("wo", [GD, D_IN], bf16, kind="ExternalInput")
    mask_d = nc.dram_tensor("mask", [128, 4 * 512], bf16, kind="ExternalInput")
    out_d = nc.dram_tensor("out", [T, D_IN], bf16, kind="ExternalOutput")

    with tile.TileContext(nc) as tc:
        with (
            tc.tile_pool(name="persist", bufs=1) as persist,
            tc.tile_pool(name="ptp", bufs=CFG_PTP) as ptp,
            tc.tile_pool(name="smalls", bufs=8) as smalls,
            tc.tile_pool(name="outsb", bufs=3) as outsb,
            tc.tile_pool(name="psum", bufs=2, space="PSUM") as psum,
        ):
            # ---- input DMAs, chunked so compute starts early ----
            wk_all = persist.tile([128, CC, HD], bf16, name="wk_all", tag="wk_all")
            wv_all = persist.tile([128, CC, HD], bf16, name="wv_all", tag="wv_all")
            wq_all = persist.tile([128, CC, GD], bf16, name="wq_all", tag="wq_all")
            xb = [
                persist.tile([128, CC, 512], bf16, name=f"xb{nq}", tag=f"xb{nq}")
                for nq in range(NQ)
            ]

            dma = nc.sync.dma_start

            def load_x(nq, c0, c1):
                dma(
                    out=xb[nq][:, c0:c1, :],
                    in_=xT_d[c0 * 128:c1 * 128, nq * 512:(nq + 1) * 512].rearrange(
                        "(c p) n -> p c n", c=c1 - c0
                    ),
                )

            def load_wq(h0, h1, c0, c1):
                dma(
                    out=wq_all[:, c0:c1, h0 * 128:h1 * 128],
                    in_=wq_d[c0 * 128:c1 * 128, h0 * 128:h1 * 128].rearrange(
                        "(c p) n -> p c n", c=c1 - c0
                    ),
                )

            def load_wk(c0, c1):
                dma(
                    out=wk_all[:, c0:c1, :],
                    in_=wk_d[:, c0 * 128:c1 * 128].rearrange(
                        "p (c n) -> p c n", c=c1 - c0
                    ),
                )

            # startup stream: K, Q(h0), Q(h1) weights rotate with the x
            # chunks in exact consumption order, so every arriving chunk
            # enables three matmuls and the stream stays PE-bound
            load_x(0, 0, 2)
            load_wk(0, 2)
            load_wq(0, 2, 0, 2)
            load_x(0, 2, 4)
            load_x(0, 4, 6)
            load_wk(2, 8)
            load_wq(0, 2, 2, 8)
            load_wk(8, 16)
            load_wq(0, 2, 8, 16)
            load_x(0, 6, 8)
            load_x(0, 8, 10)
            load_x(0, 10, 12)
            load_x(0, 12, 14)
            load_x(0, 14, 16)
            load_wq(2, 4, 0, 16)
            nc.sync.dma_start(
                out=wv_all, in_=wv_d.ap().rearrange("p (c n) -> p c n", c=CC)
            )
            mask_sb = persist.tile([128, 4 * 512], bf16, name="mask_sb", tag="mask_sb")
            nc.sync.dma_start(out=mask_sb, in_=mask_d[:, :])
            load_x(1, 0, 16)
            wo_all = persist.tile([128, GH, D_IN], bf16, name="wo_all", tag="wo_all")
            nc.sync.dma_start(
                out=wo_all, in_=wo_d.ap().rearrange("(h p) n -> p h n", h=GH)
            )
            load_x(2, 0, 16)
            load_x(3, 0, 16)

            identity = persist.tile([128, 128], bf16, name="identity", tag="identity")
            make_identity(nc, identity)
            # p-state warmup: the tensor engine clock ramps with sustained
            # use and the first real matmul waits ~3us for DMA anyway, so
            # burn idle cycles on dummy matmuls to hit full clock by then
            for i in range(CFG_WARM):
                warm = psum.tile([128, 128], f32, name=f"warm{i}", tag="psC",
                                 bufs=2)
                nc.tensor.matmul(warm, identity, identity, start=True, stop=True)

            kT_blk = [
                persist.tile([128, 512], bf16, name=f"kT{nq}", tag=f"kT{nq}")
                for nq in range(NQ)
            ]
            qT_blk = [
                [
                    persist.tile([128, 512], bf16, name=f"qT{h}_{nq}", tag=f"qT{h}_{nq}")
                    for nq in range(NQ)
                ]
                for h in range(GH)
            ]
            vext = [
                persist.tile([128, 132], bf16, name=f"vx{t}", tag=f"vx{t}")
                for t in range(TT)
            ]
            for t in range(TT):
                nc.vector.memset(vext[t][:, 128:129], 1.0)
            ctxT_blk = [
                [
                    persist.tile([128, 512], bf16, name=f"cT{h}_{nq}", tag=f"cT{h}_{nq}")
                    for nq in range(NQ)
                ]
                for h in range(GH)
            ]

            def emit_q(nq, h):
                pq = psum.tile([128, 512], f32, name=f"psq{h}", tag="psP", bufs=2)
                for c in range(CC):
                    nc.tensor.matmul(
                        pq, wq_all[:, c, h * 128:(h + 1) * 128], xb[nq][:, c, :],
                        start=(c == 0), stop=(c == CC - 1),
                    )
                nc.vector.tensor_copy(out=qT_blk[h][nq], in_=pq)

            def emit_proj(nq):
                # K, Q(h0), Q(h1) first so the 01 head-pair's scores can
                # start after three groups; V + Q(h2), Q(h3) follow
                ps = psum.tile([128, 512], f32, name="pskt", tag="psP", bufs=2)
                for c in range(CC):
                    nc.tensor.matmul(
                        ps, wk_all[:, c, :], xb[nq][:, c, :],
                        start=(c == 0), stop=(c == CC - 1),
                    )
                nc.vector.tensor_copy(out=kT_blk[nq], in_=ps)
                emit_q(nq, 0)
                emit_q(nq, 1)
                # V: one 16-chunk group per row tile (a start bit zeroes a
                # whole 2KB psum region, so groups cannot share a bank)
                for ts in range(4):
                    pv = psum.tile([128, 128], f32, name="psv", tag="psP", bufs=2)
                    for c in range(CC):
                        nc.tensor.matmul(
                            pv,
                            xb[nq][:, c, ts * 128:(ts + 1) * 128],
                            wv_all[:, c, :],
                            start=(c == 0), stop=(c == CC - 1),
                        )
                    nc.vector.tensor_copy(
                        out=vext[nq * 4 + ts][:, 0:128], in_=pv
                    )
                emit_q(nq, 2)
                emit_q(nq, 3)

            def emit_outproj_tt(tt, last=False):
                osb = outsb.tile([128, D_IN], bf16, name="osb", tag="osb")
                for nch in range(NQ):
                    po = psum.tile(
                        [128, 512], f32, name="pso", tag="psP", bufs=2
                    )
                    for h in range(GH):
                        nc.tensor.matmul(
                            po,
                            ctxT_blk[h][tt // 4][:, (tt % 4) * 128:(tt % 4 + 1) * 128],
                            wo_all[:, h, nch * 512:(nch + 1) * 512],
                            start=(h == 0), stop=(h == GH - 1),
                        )
                    if last:
                        # final tile: low-latency path — alternate DVE/Act
                        # copies (parallel engines) + per-chunk DMAs so the
                        # post-PE drain is as short as possible
                        cp = nc.vector.tensor_copy if nch % 2 == 0 else (
                            lambda out, in_: nc.scalar.copy(out=out, in_=in_))
                        cp(out=osb[:, nch * 512:(nch + 1) * 512], in_=po)
                        nc.sync.dma_start(
                            out=out_d[tt * 128:(tt + 1) * 128,
                                      nch * 512:(nch + 1) * 512],
                            in_=osb[:, nch * 512:(nch + 1) * 512],
                        )
                    else:
                        nc.vector.tensor_copy(
                            out=osb[:, nch * 512:(nch + 1) * 512], in_=po
                        )
                if not last:
                    nc.sync.dma_start(
                        out=out_d[tt * 128:(tt + 1) * 128, 0:1024], in_=osb[:, 0:1024]
                    )
                    nc.sync.dma_start(
                        out=out_d[tt * 128:(tt + 1) * 128, 1024:2048],
                        in_=osb[:, 1024:2048],
                    )

            def emit_attn(qc, fill_tts=(), interleave_outproj=False):
                """fill_tts: out-projection row tiles of the PREVIOUS block,
                emitted between this block's score steps so the psP ring
                order matches temporal order and the scheduler has ready PE
                work during the Act-paced stretches."""
                nkt = 4 * qc + 4
                fill = list(fill_tts)
                steps = 2 * nkt
                quota = len(fill) / steps if steps else 0.0
                emitted = 0.0
                step = 0
                for hp in range(2):  # heads (2hp, 2hp+1) share score tiles
                    h0 = 2 * hp
                    pt_ref = []
                    for kt in range(nkt):
                        step += 1
                        emitted += quota
                        while fill and emitted >= 1.0:
                            emit_outproj_tt(fill.pop(0))
                            emitted -= 1.0
                        # diagonal tiles: columns j < oi*128 are fully masked;
                        # compute only the live suffix [oi*128, 512)
                        oi = max(kt - 4 * qc, 0)
                        off = oi * 128
                        pss = psum.tile(
                            [128, 2, 512], f32, name="pss", tag="psS", bufs=2
                        )
                        pt = ptp.tile([128, 2, 512], bf16, name="pt", tag="pt")
                        ksl = kT_blk[kt // 4][:, (kt % 4) * 128:(kt % 4 + 1) * 128]
                        for hs in range(2):
                            nc.tensor.matmul(
                                pss[:, hs, off:512],
                                ksl,
                                qT_blk[h0 + hs][qc][:, off:512],
                                start=True, stop=True,
                            )
                        # one exp for both heads' tiles
                        nc.scalar.activation(
                            out=pt[:, :, off:512], in_=pss[:, :, off:512],
                            func=mybir.ActivationFunctionType.Exp, scale=SCALE,
                        )
                        if kt >= 4 * qc:  # triangular mask on the partial block
                            tri = mask_sb[:, oi * 512 + off:oi * 512 + off + 128]
                            for hs in range(2):
                                nc.vector.tensor_mul(
                                    pt[:, hs, off:off + 128],
                                    pt[:, hs, off:off + 128], tri,
                                )
                        pt_ref.append(pt)
                    # last block: reverse the sub order so the final-closing
                    # ctx groups already have every exp tile resident and the
                    # tail collapses to one chain + one out-proj row tile
                    subs = (3, 2, 1, 0) if interleave_outproj else (0, 1, 2, 3)
                    for sub in subs:
                        qi = qc * 4 + sub
                        for hs in range(2):
                            h = h0 + hs
                            cps = psum.tile(
                                [128, 132], f32, name="cps", tag="psC", bufs=2
                            )
                            for kt in range(qi + 1):
                                nc.tensor.matmul(
                                    cps[:, 0:129],
                                    pt_ref[kt][:, hs, sub * 128:(sub + 1) * 128],
                                    vext[kt][:, 0:129],
                                    start=(kt == 0), stop=(kt == qi),
                                )
                            rec = smalls.tile([128, 1], f32, name="rec", tag="rec")
                            nc.vector.reciprocal(rec, cps[:, 128:129])
                            cn = smalls.tile([128, 128], bf16, name="cn", tag="cn")
                            nc.vector.tensor_scalar_mul(cn, cps[:, 0:128], rec)
                            tp = psum.tile(
                                [128, 128], bf16, name="tp", tag="psC", bufs=2
                            )
                            nc.tensor.transpose(tp[:, 0:128], cn, identity)
                            nc.vector.tensor_copy(
                                out=ctxT_blk[h][qc][:, sub * 128:(sub + 1) * 128],
                                in_=tp[:, 0:128],
                            )
                        if interleave_outproj and hp == 1:
                            # all four heads' ctxT for this sub are now
                            # emitted: the matching out-projection row tile
                            # can stream right behind the softmax chains
                            emit_outproj_tt(4 * qc + sub, last=(sub == 3))
                for tt in fill:
                    emit_outproj_tt(tt)

            # Emission order doubles as scheduler priority AND psP ring
            # order: out-projections of block N are woven into block N+1's
            # attention so the ring order matches temporal order and the
            # Act-paced stretches always have ready PE work.
            emit_proj(0)
            emit_attn(0)
            emit_proj(1)
            emit_attn(1, fill_tts=range(0, 4))
            emit_proj(2)
            emit_attn(2, fill_tts=range(4, 8))
            emit_proj(3)
            emit_attn(3, fill_tts=range(8, 12), interleave_outproj=True)

    nc.compile()
    return nc


def _get_compiled():
    global _COMPILED
    if _COMPILED is None:
        _COMPILED = _build()
    return _COMPILED


def _causal_mask():
    i = np.arange(128)[:, None]
    j = np.arange(512)[None, :]
    return np.concatenate(
        [(oi * 128 + i <= j) for oi in range(4)], axis=1
    ).astype(ml_dtypes.bfloat16)


def _pack_pmajor(w):
    # [CC*128, HD] -> [128, CC*HD]: out[p, c*HD+d] = w[c*128+p, d]
    return np.ascontiguousarray(
        w.reshape(CC, 128, -1).transpose(1, 0, 2).reshape(128, -1)
    )


def make_in_maps(x, Wq, Wk, Wv, Wo):
    bf16 = ml_dtypes.bfloat16
    x = np.asarray(x, np.float32)
    Wq = np.asarray(Wq, np.float32)
    Wk = np.asarray(Wk, np.float32)
    Wv = np.asarray(Wv, np.float32)
    Wo = np.asarray(Wo, np.float32)
    mask = _causal_mask()
    in_maps = []
    for core in range(8):
        bi, g = divmod(core, N_KV)
        in_maps.append({
            "xT": np.ascontiguousarray(x[bi].T).astype(bf16),
            "wq": np.ascontiguousarray(Wq[:, g * GD:(g + 1) * GD]).astype(bf16),
            "wk": _pack_pmajor(Wk[:, g * HD:(g + 1) * HD]).astype(bf16),
            "wv": _pack_pmajor(Wv[:, g * HD:(g + 1) * HD]).astype(bf16),
            "wo": np.ascontiguousarray(Wo[g * GD:(g + 1) * GD, :]).astype(bf16),
            "mask": mask,
        })
    return in_maps


def kernel(x, Wq, Wk, Wv, Wo, bo):
    from concourse.bass_utils import run_bass_kernel_spmd

    nc = _get_compiled()
    in_maps = make_in_maps(x, Wq, Wk, Wv, Wo)
    res = run_bass_kernel_spmd(nc, in_maps, core_ids=list(range(8)))
    out = np.zeros((B, T, D_IN), np.float32)
    for core in range(8):
        out[core // N_KV] += res.results[core]["out"]
    out += np.asarray(bo, np.float32)
    return out


# revision 60
# speedup vs baseline: 1.0015x; 1.0015x over previous
"""GQA attention block (16 query heads / 4 KV groups, head_dim 128) on 8 TRN2 NeuronCores.

Sharding: data-parallel over batch (b=2) x tensor-parallel over the 4 KV groups.
Core c handles batch c//4, KV group c%4 (4 query heads). Each core computes its
group's Q/K/V projections, causal softmax attention, and a partial out-projection
(row-shard of Wo); the host sums the 4 partials per batch and adds the bias.

All matmuls run in bf16 (fp32 PSUM accumulation). Host pre-transposes x to x^T
(and packs Wk/Wv partition-major) so every matmul operand is already in the
[K, M]/[K, N] layout the PE wants; the only on-chip transposes are the per-block
128x128 context-tile transposes ahead of the out-projection. Softmax runs without
the running-max (score scale is bounded by the input distribution); the denominator
comes from a ones-column appended to V.

Schedule highlights:
- Startup DMAs rotate wk/wq(h0,h1)/x chunks in exact consumption order so
  K, Q(h0), Q(h1) stream behind the DMA from ~3us; dummy identity matmuls
  warm the PE p-state during the initial DMA latency so real matmuls run at
  full clock (the cost model halves PE speed until ~3us of sustained use).
- Scores for the two heads of a GQA head-pair go into one 2-bank PSUM tile
  [128, 2, 512] and share a single Exp activation per k-tile: 80 exps total
  instead of 160 (the Act engine paced the attention endgame before).
- Projection and out-projection groups share one 2-buf PSUM tag, and the
  out-projections of block N are woven into block N+1's attention emission:
  ring order matches temporal order while the scheduler's ready-heap pops
  them exactly in the Act-paced stalls. The last block's out-projections are
  emitted per sub-tile right behind the softmax chains, with a low-latency
  DVE-copy + per-chunk-DMA path on the final row tile to shorten the drain.
"""

import math
import os

import ml_dtypes
import numpy as np

B = 2
T = 2048
D_IN = 2048
N_KV = 4          # KV groups (one per core within a batch)
GH = 4            # query heads per KV group
HD = 128          # head dim
GD = GH * HD      # 512: per-group q/ctx width
TT = T // 128     # 16 row tiles
CC = D_IN // 128  # 16 contraction chunks
NQ = T // 512     # 4 query chunks of 512
SCALE = 1.0 / math.sqrt(HD)

CFG_PTP = int(os.environ.get("K_PTP", "22"))
CFG_WARM = int(os.environ.get("K_WARM", "14"))

_COMPILED = None


def _build():
    import concourse.bacc as bacc
    import concourse.tile as tile
    from concourse import mybir
    from concourse.masks import make_identity

    bf16 = mybir.dt.bfloat16
    f32 = mybir.dt.float32

    nc = bacc.Bacc("TRN2", target_bir_lowering=False, debug=False)

    # xT: x^T per batch; wk/wv packed partition-major on host: [128, c*HD]
    xT_d = nc.dram_tensor("xT", [D_IN, T], bf16, kind="ExternalInput")
    wq_d = nc.dram_tensor("wq", [D_IN, GD], bf16, kind="ExternalInput")
    wk_d = nc.dram_tensor("wk", [128, CC * HD], bf16, kind="ExternalInput")
    wv_d = nc.dram_tensor("wv", [128, CC * HD], bf16, kind="ExternalInput")
    wo_d = nc.dram_tensor("wo", [GD, D_IN], bf16, kind="ExternalInput")
    mask_d = nc.dram_tensor("mask", [128, 4 * 512], bf16, kind="ExternalInput")
    out_d = nc.dram_tensor("out", [T, D_IN], bf16, kind="ExternalOutput")

    with tile.TileContext(nc) as tc:
        with (
            tc.tile_pool(name="persist", bufs=1) as persist,
            tc.tile_pool(name="ptp", bufs=CFG_PTP) as ptp,
            tc.tile_pool(name="smalls", bufs=8) as smalls,
            tc.tile_pool(name="outsb", bufs=3) as outsb,
            tc.tile_pool(name="psum", bufs=2, space="PSUM") as psum,
        ):
            # ---- input DMAs, chunked so compute starts early ----
            wk_all = persist.tile([128, CC, HD], bf16, name="wk_all", tag="wk_all")
            wv_all = persist.tile([128, CC, HD], bf16, name="wv_all", tag="wv_all")
            wq_all = persist.tile([128, CC, GD], bf16, name="wq_all", tag="wq_all")
            xb = [
                persist.tile([128, CC, 512], bf16, name=f"xb{nq}", tag=f"xb{nq}")
                for nq in range(NQ)
            ]

            dma = nc.sync.dma_start

            def load_x(nq, c0, c1):
                dma(
                    out=xb[nq][:, c0:c1, :],
                    in_=xT_d[c0 * 128:c1 * 128, nq * 512:(nq + 1) * 512].rearrange(
                        "(c p) n -> p c n", c=c1 - c0
                    ),
                )

            def load_wq(h0, h1, c0, c1):
                dma(
                    out=wq_all[:, c0:c1, h0 * 128:h1 * 128],
                    in_=wq_d[c0 * 128:c1 * 128, h0 * 128:h1 * 128].rearrange(
                        "(c p) n -> p c n", c=c1 - c0
                    ),
                )

            def load_wk(c0, c1):
                dma(
                    out=wk_all[:, c0:c1, :],
                    in_=wk_d[:, c0 * 128:c1 * 128].rearrange(
                        "p (c n) -> p c n", c=c1 - c0
                    ),
                )

            # startup stream: K, Q(h0), Q(h1) weights rotate with the x
            # chunks in exact consumption order, so every arriving chunk
            # enables three matmuls and the stream stays PE-bound
            load_x(0, 0, 2)
            load_wk(0, 2)
            load_wq(0, 2, 0, 2)
            load_x(0, 2, 4)
            load_x(0, 4, 6)
            load_wk(2, 8)
            load_wq(0, 2, 2, 8)
            load_wk(8, 16)
            load_x(0, 6, 8)
            load_x(0, 8, 10)
            load_wq(0, 2, 8, 12)
            load_x(0, 10, 12)
            load_x(0, 12, 14)
            load_wq(0, 2, 12, 16)
            load_x(0, 14, 16)
            load_wq(2, 4, 0, 16)
            nc.sync.dma_start(
                out=wv_all, in_=wv_d.ap().rearrange("p (c n) -> p c n", c=CC)
            )
            mask_sb = persist.tile([128, 4 * 512], bf16, name="mask_sb", tag="mask_sb")
            nc.sync.dma_start(out=mask_sb, in_=mask_d[:, :])
            load_x(1, 0, 16)
            wo_all = persist.tile([128, GH, D_IN], bf16, name="wo_all", tag="wo_all")
            nc.sync.dma_start(
                out=wo_all, in_=wo_d.ap().rearrange("(h p) n -> p h n", h=GH)
            )
            load_x(2, 0, 16)
            load_x(3, 0, 16)

            identity = persist.tile([128, 128], bf16, name="identity", tag="identity")
            make_identity(nc, identity)
            # p-state warmup: the tensor engine clock ramps with sustained
            # use and the first real matmul waits ~3us for DMA anyway, so
            # burn idle cycles on dummy matmuls to hit full clock by then
            for i in range(CFG_WARM):
                warm = psum.tile([128, 128], f32, name=f"warm{i}", tag="psC",
                                 bufs=2)
                nc.tensor.matmul(warm, identity, identity, start=True, stop=True)

            kT_blk = [
                persist.tile([128, 512], bf16, name=f"kT{nq}", tag=f"kT{nq}")
                for nq in range(NQ)
            ]
            qT_blk = [
                [
                    persist.tile([128, 512], bf16, name=f"qT{h}_{nq}", tag=f"qT{h}_{nq}")
                    for nq in range(NQ)
                ]
                for h in range(GH)
            ]
            vext = [
                persist.tile([128, 132], bf16, name=f"vx{t}", tag=f"vx{t}")
                for t in range(TT)
            ]
            for t in range(TT):
                nc.vector.memset(vext[t][:, 128:129], 1.0)
            ctxT_blk = [
                [
                    persist.tile([128, 512], bf16, name=f"cT{h}_{nq}", tag=f"cT{h}_{nq}")
                    for nq in range(NQ)
                ]
                for h in range(GH)
            ]

            def emit_q(nq, h):
                pq = psum.tile([128, 512], f32, name=f"psq{h}", tag="psP", bufs=2)
                for c in range(CC):
                    nc.tensor.matmul(
                        pq, wq_all[:, c, h * 128:(h + 1) * 128], xb[nq][:, c, :],
                        start=(c == 0), stop=(c == CC - 1),
                    )
                nc.vector.tensor_copy(out=qT_blk[h][nq], in_=pq)

            def emit_proj(nq):
                # K, Q(h0), Q(h1) first so the 01 head-pair's scores can
                # start after three groups; V + Q(h2), Q(h3) follow
                ps = psum.tile([128, 512], f32, name="pskt", tag="psP", bufs=2)
                for c in range(CC):
                    nc.tensor.matmul(
                        ps, wk_all[:, c, :], xb[nq][:, c, :],
                        start=(c == 0), stop=(c == CC - 1),
                    )
                nc.vector.tensor_copy(out=kT_blk[nq], in_=ps)
                emit_q(nq, 0)
                emit_q(nq, 1)
                # V: one 16-chunk group per row tile (a start bit zeroes a
                # whole 2KB psum region, so groups cannot share a bank)
                for ts in range(4):
                    pv = psum.tile([128, 128], f32, name="psv", tag="psP", bufs=2)
                    for c in range(CC):
                        nc.tensor.matmul(
                            pv,
                            xb[nq][:, c, ts * 128:(ts + 1) * 128],
                            wv_all[:, c, :],
                            start=(c == 0), stop=(c == CC - 1),
                        )
                    nc.vector.tensor_copy(
                        out=vext[nq * 4 + ts][:, 0:128], in_=pv
                    )
                emit_q(nq, 2)
                emit_q(nq, 3)

            def emit_outproj_tt(tt, last=False):
                osb = outsb.tile([128, D_IN], bf16, name="osb", tag="osb")
                for nch in range(NQ):
                    po = psum.tile(
                        [128, 512], f32, name="pso", tag="psP", bufs=2
                    )
                    for h in range(GH):
                        nc.tensor.matmul(
                            po,
                            ctxT_blk[h][tt // 4][:, (tt % 4) * 128:(tt % 4 + 1) * 128],
                            wo_all[:, h, nch * 512:(nch + 1) * 512],
                            start=(h == 0), stop=(h == GH - 1),
                        )
                    if last:
                        # final tile: low-latency path — alternate DVE/Act
                        # copies (parallel engines) + per-chunk DMAs so the
                        # post-PE drain is as short as possible
                        cp = nc.vector.tensor_copy if nch % 2 == 0 else (
                            lambda out, in_: nc.scalar.copy(out=out, in_=in_))
                        cp(out=osb[:, nch * 512:(nch + 1) * 512], in_=po)
                        nc.sync.dma_start(
                            out=out_d[tt * 128:(tt + 1) * 128,
                                      nch * 512:(nch + 1) * 512],
                            in_=osb[:, nch * 512:(nch + 1) * 512],
                        )
                    else:
                        nc.vector.tensor_copy(
                            out=osb[:, nch * 512:(nch + 1) * 512], in_=po
                        )
                if not last:
                    nc.sync.dma_start(
                        out=out_d[tt * 128:(tt + 1) * 128, 0:1024], in_=osb[:, 0:1024]
                    )
                    nc.sync.dma_start(
                        out=out_d[tt * 128:(tt + 1) * 128, 1024:2048],
                        in_=osb[:, 1024:2048],
                    )

            def emit_attn(qc, fill_tts=(), interleave_outproj=False):
                """fill_tts: out-projection row tiles of the PREVIOUS block,
                emitted between this block's score steps so the psP ring
                order matches temporal order and the scheduler has ready PE
                work during the Act-paced stretches."""
                nkt = 4 * qc + 4
                fill = list(fill_tts)
                steps = 2 * nkt
                quota = len(fill) / steps if steps else 0.0
                emitted = 0.0
                step = 0
                for hp in range(2):  # heads (2hp, 2hp+1) share score tiles
                    h0 = 2 * hp
                    pt_ref = []
                    for kt in range(nkt):
                        step += 1
                        emitted += quota
                        while fill and emitted >= 1.0:
                            emit_outproj_tt(fill.pop(0))
                            emitted -= 1.0
                        # diagonal tiles: columns j < oi*128 are fully masked;
                        # compute only the live suffix [oi*128, 512)
                        oi = max(kt - 4 * qc, 0)
                        off = oi * 128
                        pss = psum.tile(
                            [128, 2, 512], f32, name="pss", tag="psS", bufs=2
                        )
                        pt = ptp.tile([128, 2, 512], bf16, name="pt", tag="pt")
                        ksl = kT_blk[kt // 4][:, (kt % 4) * 128:(kt % 4 + 1) * 128]
                        for hs in range(2):
                            nc.tensor.matmul(
                                pss[:, hs, off:512],
                                ksl,
                                qT_blk[h0 + hs][qc][:, off:512],
                                start=True, stop=True,
                            )
                        # one exp for both heads' tiles
                        nc.scalar.activation(
                            out=pt[:, :, off:512], in_=pss[:, :, off:512],
                            func=mybir.ActivationFunctionType.Exp, scale=SCALE,
                        )
                        if kt >= 4 * qc:  # triangular mask on the partial block
                            tri = mask_sb[:, oi * 512 + off:oi * 512 + off + 128]
                            for hs in range(2):
                                nc.vector.tensor_mul(
                                    pt[:, hs, off:off + 128],
                                    pt[:, hs, off:off + 128], tri,
                                )
                        pt_ref.append(pt)
                    # last block: reverse the sub order so the final-closing
                    # ctx groups already have every exp tile resident and the
                    # tail collapses to one chain + one out-proj row tile
                    subs = (3, 2, 1, 0) if interleave_outproj else (0, 1, 2, 3)
                    for sub in subs:
                        qi = qc * 4 + sub
                        for hs in range(2):
                            h = h0 + hs
                            cps = psum.tile(
                                [128, 132], f32, name="cps", tag="psC", bufs=2
                            )
                            for kt in range(qi + 1):
                                nc.tensor.matmul(
                                    cps[:, 0:129],
                                    pt_ref[kt][:, hs, sub * 128:(sub + 1) * 128],
                                    vext[kt][:, 0:129],
                                    start=(kt == 0), stop=(kt == qi),
                                )
                            rec = smalls.tile([128, 1], f32, name="rec", tag="rec")
                            nc.vector.reciprocal(rec, cps[:, 128:129])
                            cn = smalls.tile([128, 128], bf16, name="cn", tag="cn")
                            nc.vector.tensor_scalar_mul(cn, cps[:, 0:128], rec)
                            tp = psum.tile(
                                [128, 128], bf16, name="tp", tag="psC", bufs=2
                            )
                            nc.tensor.transpose(tp[:, 0:128], cn, identity)
                            nc.vector.tensor_copy(
                                out=ctxT_blk[h][qc][:, sub * 128:(sub + 1) * 128],
                                in_=tp[:, 0:128],
                            )
                        if interleave_outproj and hp == 1:
                            # all four heads' ctxT for this sub are now
                            # emitted: the matching out-projection row tile
                            # can stream right behind the softmax chains
                            emit_outproj_tt(4 * qc + sub, last=(sub == 3))
                for tt in fill:
                    emit_outproj_tt(tt)

            # Emission order doubles as scheduler priority AND psP ring
            # order: out-projections of block N are woven into block N+1's
            # attention so the ring order matches temporal order and the
            # Act-paced stretches always have ready PE work.
            emit_proj(0)
            emit_attn(0)
            emit_proj(1)
            emit_attn(1, fill_tts=range(0, 4))
            emit_proj(2)
            emit_attn(2, fill_tts=range(4, 8))
            emit_proj(3)
            emit_attn(3, fill_tts=range(8, 12), interleave_outproj=True)

    nc.compile()
    return nc


def _get_compiled():
    global _COMPILED
    if _COMPILED is None:
        _COMPILED = _build()
    return _COMPILED


def _causal_mask():
    i = np.arange(128)[:, None]
    j = np.arange(512)[None, :]
    return np.concatenate(
        [(oi * 128 + i <= j) for oi in range(4)], axis=1
    ).astype(ml_dtypes.bfloat16)


def _pack_pmajor(w):
    # [CC*128, HD] -> [128, CC*HD]: out[p, c*HD+d] = w[c*128+p, d]
    return np.ascontiguousarray(
        w.reshape(CC, 128, -1).transpose(1, 0, 2).reshape(128, -1)
    )


def make_in_maps(x, Wq, Wk, Wv, Wo):
    bf16 = ml_dtypes.bfloat16
    x = np.asarray(x, np.float32)
    Wq = np.asarray(Wq, np.float32)
    Wk = np.asarray(Wk, np.float32)
    Wv = np.asarray(Wv, np.float32)
    Wo = np.asarray(Wo, np.float32)
    mask = _causal_mask()
    in_maps = []
    for core in range(8):
        bi, g = divmod(core, N_KV)
        in_maps.append({
            "xT": np.ascontiguousarray(x[bi].T).astype(bf16),
            "wq": np.ascontiguousarray(Wq[:, g * GD:(g + 1) * GD]).astype(bf16),
            "wk": _pack_pmajor(Wk[:, g * HD:(g + 1) * HD]).astype(bf16),
            "wv": _pack_pmajor(Wv[:, g * HD:(g + 1) * HD]).astype(bf16),
            "wo": np.ascontiguousarray(Wo[g * GD:(g + 1) * GD, :]).astype(bf16),
            "mask": mask,
        })
    return in_maps


def kernel(x, Wq, Wk, Wv, Wo, bo):
    from concourse.bass_utils import run_bass_kernel_spmd

    nc = _get_compiled()
    in_maps = make_in_maps(x, Wq, Wk, Wv, Wo)
    res = run_bass_kernel_spmd(nc, in_maps, core_ids=list(range(8)))
    out = np.zeros((B, T, D_IN), np.float32)
    for core in range(8):
        out[core // N_KV] += res.results[core]["out"]
    out += np.asarray(bo, np.float32)
    return out
